# revision 7
# baseline (speedup 1.0000x reference)
"""Correlation cost-volume kernel for Trainium2 (8 NeuronCores, data-parallel over batch).

cost[b, d, h, w] = mean_c left[b, c, h, w] * right[b, c, h, w - d]   (0 for w < d)

Per (b, h) this is the 48-wide band of the Gram matrix G = L^T R (K = c = 128).
Pipeline per h-group of HC rows:
  bf16 Gram matmuls (PE) -> PSUM (4 h per 2-bank tile, bufs=3)
  -> scaled copy to bf16 X[i, f*HC + hh] (ACT/DVE alternate, h-interleaved)
  -> shear DMA: S[i, dd*HC + hh] = X[i, (i+dd)*HC + hh]  (flat fused-step AP,
     127+1 partition split to avoid the HW DGE fused-step bug)
  -> (next loop iteration, so PE never stalls on the shear) PE transposes of
     128-col S slices -> PSUM [128, m] -> copy to O fp32
  -> DMA to out[d, h, w] (one DMA per dd_l in 0..3, negative d-stride in dim1).
Input loads are split into 8-row chunks so the first matmuls start early.
"""

import sys
from contextlib import ExitStack

import numpy as np

if "/opt/trn_rl_repo" not in sys.path:
    sys.path.insert(0, "/opt/trn_rl_repo")

import concourse.bass as bass
import concourse.mybir as mybir
from concourse import bacc, tile
from concourse.ap import AP

B = 8
C = 128
H = 160
W = 320
D = 48
PAD = D - 1  # 47
HC = 32  # h rows per group
FW = PAD + 128  # 175, X f-slots per h row
MDT = mybir.dt.bfloat16  # matmul input dtype
SDT = mybir.dt.bfloat16  # X/S (post-mean) dtype

# w-blocks: (wb, M)
WBLOCKS = [(0, 128), (128, 128), (256, 64)]
HCHUNK = 8  # h rows per input DMA chunk


def _ncols(wb, m):
    w0 = max(0, wb - PAD)
    return min(W, wb + m) - w0, w0


def build_nc(h=H):
    nc = bacc.Bacc("TRN2", target_bir_lowering=False, debug=False)
    left_d = nc.dram_tensor("left", [C, h, W], mybir.dt.float32, kind="ExternalInput")
    right_d = nc.dram_tensor("right", [C, h, W], mybir.dt.float32, kind="ExternalInput")
    ident_d = nc.dram_tensor("ident", [128, 128], mybir.dt.float32, kind="ExternalInput")
    out_d = nc.dram_tensor("out", [D, h, W], mybir.dt.float32, kind="ExternalOutput")

    ngroups = h // HC
    hw = h * W

    with tile.TileContext(nc) as tc, ExitStack() as ctx:
        const_pool = ctx.enter_context(tc.tile_pool(name="const", bufs=1))
        lr_pool = ctx.enter_context(tc.tile_pool(name="lr", bufs=2))
        x_pool = ctx.enter_context(tc.tile_pool(name="x", bufs=3))
        s_pool = ctx.enter_context(tc.tile_pool(name="s", bufs=2))
        o_pool = ctx.enter_context(tc.tile_pool(name="o", bufs=2))
        g_pool = ctx.enter_context(tc.tile_pool(name="g", bufs=3, space="PSUM"))
        t_pool = ctx.enter_context(tc.tile_pool(name="t", bufs=2, space="PSUM"))

        ident = const_pool.tile([128, 128], SDT)
        nc.gpsimd.dma_start(ident[:], ident_d[:])

        # persistent X tiles for wb=0 (the only block with a zero prefix);
        # the f < 47 region is zeroed once and never overwritten, so the
        # per-group memset (which would block the SWDGE input queue) goes away
        x0_tiles = [
            const_pool.tile([128, HC * FW], SDT, tag=f"X0{i}", name=f"x0_{i}")
            for i in (0, 1)
        ]
        for x0 in x0_tiles:
            nc.gpsimd.memset(x0[:, : PAD * HC], 0.0)

        copy_parity = 0
        # deferred per-group state: (stiles, otile, h0) for the transpose pass
        pending = None

        def emit_transposes(stiles, otile, h0):
            nonlocal copy_parity
            for wb, m in WBLOCKS:
                stile = stiles[wb]
                for tq in range(3):
                    tt = t_pool.tile([128, 4 * m], SDT, tag="T")
                    for s in range(4):
                        a = 4 * tq + s
                        nc.tensor.transpose(
                            tt[:, s * m : (s + 1) * m],
                            stile[:, a * 128 : (a + 1) * 128],
                            ident[:m, :m],
                        )
                    # O[p, a*W + wb + i] <- T[p, (a-4*tq)*m + i]
                    dst = AP(
                        otile[:].tensor,
                        otile[:].offset + 4 * tq * W + wb,
                        [[12 * W, 128], [W, 4], [1, m]],
                    )
                    src = tt[:].rearrange("p (a i) -> p a i", i=m)
                    if copy_parity & 1:
                        nc.scalar.copy(dst, src)
                    else:
                        nc.vector.tensor_copy(dst, src)
                    copy_parity += 1
            # out[47-(4a+dl), h0+hh, w] <- O[dl*32+hh, a*W+w]
            for dl in range(4):
                dst = AP(
                    out_d,
                    (PAD - dl) * hw + h0 * W,
                    [[W, HC], [-4 * hw, 12], [1, W]],
                )
                src = otile[dl * HC : (dl + 1) * HC, :].rearrange(
                    "p (a w) -> p a w", w=W
                )
                nc.sync.dma_start(dst, src)

        for g in range(ngroups):
            h0 = g * HC
            ltile = lr_pool.tile([C, HC * W], MDT, tag="L")
            rtile = lr_pool.tile([C, HC * W], MDT, tag="R")
            # SWDGE DMA with fp32 -> bf16 cast, split into HCHUNK-row pieces
            for k in range(HC // HCHUNK):
                nc.gpsimd.dma_start(
                    ltile[:, k * HCHUNK * W : (k + 1) * HCHUNK * W].rearrange(
                        "p (a b) -> p a b", a=HCHUNK
                    ),
                    left_d[:, h0 + k * HCHUNK : h0 + (k + 1) * HCHUNK, :],
                )
                nc.gpsimd.dma_start(
                    rtile[:, k * HCHUNK * W : (k + 1) * HCHUNK * W].rearrange(
                        "p (a b) -> p a b", a=HCHUNK
                    ),
                    right_d[:, h0 + k * HCHUNK : h0 + (k + 1) * HCHUNK, :],
                )

            # O spans all 3 w-blocks so out-DMA runs are full 1280B w rows
            otile = o_pool.tile([128, 12 * W], mybir.dt.float32, tag="O")
            stiles = {}

            for wb, m in WBLOCKS:
                ncols, w0 = _ncols(wb, m)
                foff = PAD - wb + w0  # 47 for wb=0 else 0
                # X is h-interleaved: X[i, f*HC + hh] = G_hh[i, f]
                if foff:
                    xtile = x0_tiles[g & 1]
                else:
                    xtile = x_pool.tile([m, HC * FW], SDT, tag="X")

                for hq in range(HC // 4):
                    # 4 matmuls per 2-bank PSUM tile at 256-slot alignment
                    gt = g_pool.tile([m, 1024], mybir.dt.float32, tag="G")
                    for s in range(4):
                        hh = 4 * hq + s
                        nc.tensor.matmul(
                            gt[:, s * 256 : s * 256 + ncols],
                            ltile[:, hh * W + wb : hh * W + wb + m],
                            rtile[:, hh * W + w0 : hh * W + w0 + ncols],
                            start=True,
                            stop=True,
                        )
                    # PSUM -> X with 1/C scaling; dst interleaved (f stride HC).
                    # hh innermost: 4 consecutive bf16 -> packed 32-bit writes
                    dst = AP(
                        xtile[:].tensor,
                        xtile[:].offset + foff * HC + 4 * hq,
                        [[HC * FW, m], [HC, ncols], [1, 4]],
                    )
                    src = AP(gt[:].tensor, gt[:].offset, [[1024, m], [1, ncols], [256, 4]])
                    if copy_parity & 1:
                        nc.scalar.mul(dst, src, 1.0 / C)
                    else:
                        nc.vector.tensor_scalar_mul(dst, src, 1.0 / C)
                    copy_parity += 1

                # shear: S[i, dd*HC + hh] = X[i, (i+dd)*HC + hh]; per-partition
                # window is one contiguous HC*D run -> 2-dim flat AP with a
                # fused (row+byte) step. HW DGE constraints: fused-step APs
                # are only correct with offset 0 and partition count not in
                # {64, 128} -> split m into (m-1) + 1; the single-partition
                # leftover is a plain rectangular DMA.
                stile = s_pool.tile([m, HC * D], SDT, tag=f"S{wb}")
                p_lo = m - 1
                # HWDGE via scalar: keeps the shear off the single SWDGE
                # queue so it can't block the input stream behind it
                nc.scalar.dma_start(
                    stile[0:p_lo, :],
                    AP(
                        xtile[:].tensor,
                        xtile[:].offset,
                        [[HC * (FW + 1), p_lo], [1, HC * D]],
                    ),
                )
                nc.scalar.dma_start(
                    stile[p_lo:m, :],
                    xtile[p_lo:m, p_lo * HC : p_lo * HC + HC * D],
                )
                stiles[wb] = stile

            # transposes of the PREVIOUS group run after this group's Gram
            # matmuls so the PE never waits on this group's shear DMAs
            if pending is not None:
                emit_transposes(*pending)
            pending = (stiles, otile, h0)

        emit_transposes(*pending)

    nc.compile()
    return nc


def kernel(left_feature: np.ndarray, right_feature: np.ndarray) -> np.ndarray:
    from concourse import bass_utils

    nc = build_nc()
    ident = np.eye(128, dtype=np.float32)
    in_maps = [
        {
            "left": np.ascontiguousarray(left_feature[b]),
            "right": np.ascontiguousarray(right_feature[b]),
            "ident": ident,
        }
        for b in range(B)
    ]
    res = bass_utils.run_bass_kernel_spmd(nc, in_maps, list(range(B)))
    return np.stack([res.results[b]["out"] for b in range(B)], axis=0)


# revision 8
# speedup vs baseline: 1.2826x; 1.2826x over previous
"""Correlation cost-volume kernel for Trainium2 (8 NeuronCores, data-parallel over batch).

cost[b, d, h, w] = mean_c left[b, c, h, w] * right[b, c, h, w - d]   (0 for w < d)

Per (b, h) this is the 48-wide band of the Gram matrix G = L^T R (K = c = 128).
Pipeline per h-group of HC rows:
  bf16 Gram matmuls (PE) -> PSUM (4 h per 2-bank tile, bufs=3)
  -> scaled copy to bf16 X[i, f*HC + hh] (ACT/DVE alternate, h-interleaved)
  -> shear DMA: S[i, dd*HC + hh] = X[i, (i+dd)*HC + hh]  (flat fused-step AP,
     127+1 partition split to avoid the HW DGE fused-step bug)
  -> (next loop iteration, so PE never stalls on the shear) PE transposes of
     128-col S slices -> PSUM [128, m] -> copy to O fp32
  -> DMA to out[d, h, w] (one DMA per dd_l in 0..3, negative d-stride in dim1).
Input loads are split into 8-row chunks so the first matmuls start early.
"""

import sys
from contextlib import ExitStack

import numpy as np

if "/opt/trn_rl_repo" not in sys.path:
    sys.path.insert(0, "/opt/trn_rl_repo")

import concourse.bass as bass
import concourse.mybir as mybir
from concourse import bacc, tile
from concourse.ap import AP

B = 8
C = 128
H = 160
W = 320
D = 48
PAD = D - 1  # 47
HC = 32  # h rows per group
FW = PAD + 128  # 175, X f-slots per h row
MDT = mybir.dt.bfloat16  # matmul input dtype
SDT = mybir.dt.bfloat16  # X/S (post-mean) dtype

# w-blocks: (wb, M)
WBLOCKS = [(0, 128), (128, 128), (256, 64)]
HCHUNK = 8  # h rows per input DMA chunk


def _ncols(wb, m):
    w0 = max(0, wb - PAD)
    return min(W, wb + m) - w0, w0


def build_nc(h=H):
    nc = bacc.Bacc("TRN2", target_bir_lowering=False, debug=False)
    left_d = nc.dram_tensor("left", [C, h, W], mybir.dt.float32, kind="ExternalInput")
    right_d = nc.dram_tensor("right", [C, h, W], mybir.dt.float32, kind="ExternalInput")
    ident_d = nc.dram_tensor("ident", [128, 128], mybir.dt.float32, kind="ExternalInput")
    out_d = nc.dram_tensor("out", [D, h, W], mybir.dt.float32, kind="ExternalOutput")

    ngroups = h // HC
    hw = h * W

    with tile.TileContext(nc) as tc, ExitStack() as ctx:
        const_pool = ctx.enter_context(tc.tile_pool(name="const", bufs=1))
        lr_pool = ctx.enter_context(tc.tile_pool(name="lr", bufs=2))
        x_pool = ctx.enter_context(tc.tile_pool(name="x", bufs=3))
        s_pool = ctx.enter_context(tc.tile_pool(name="s", bufs=2))
        o_pool = ctx.enter_context(tc.tile_pool(name="o", bufs=2))
        g_pool = ctx.enter_context(tc.tile_pool(name="g", bufs=3, space="PSUM"))
        t_pool = ctx.enter_context(tc.tile_pool(name="t", bufs=2, space="PSUM"))

        ident = const_pool.tile([128, 128], SDT)
        nc.gpsimd.dma_start(ident[:], ident_d[:])

        # persistent X tiles for wb=0 (the only block with a zero prefix);
        # the f < 47 region is zeroed once and never overwritten, so the
        # per-group memset (which would block the SWDGE input queue) goes away
        x0_tiles = [
            const_pool.tile([128, HC * FW], SDT, tag=f"X0{i}", name=f"x0_{i}")
            for i in (0, 1)
        ]
        for x0 in x0_tiles:
            nc.gpsimd.memset(x0[:, : PAD * HC], 0.0)

        copy_parity = 0
        # deferred per-group state: (stiles, otile, h0) for the transpose pass
        pending = None

        def emit_transposes(stiles, otile, h0):
            nonlocal copy_parity
            for wb, m in WBLOCKS:
                stile = stiles[wb]
                for tq in range(3):
                    tt = t_pool.tile([128, 4 * m], SDT, tag="T")
                    for s in range(4):
                        a = 4 * tq + s
                        nc.tensor.transpose(
                            tt[:, s * m : (s + 1) * m],
                            stile[:, a * 128 : (a + 1) * 128],
                            ident[:m, :m],
                        )
                    # O[p, a*W + wb + i] <- T[p, (a-4*tq)*m + i]
                    dst = AP(
                        otile[:].tensor,
                        otile[:].offset + 4 * tq * W + wb,
                        [[12 * W, 128], [W, 4], [1, m]],
                    )
                    src = tt[:].rearrange("p (a i) -> p a i", i=m)
                    if copy_parity & 1:
                        nc.scalar.copy(dst, src)
                    else:
                        nc.vector.tensor_copy(dst, src)
                    copy_parity += 1
            # out[47-(4a+dl), h0+hh, w] <- O[dl*32+hh, a*W+w]
            for dl in range(4):
                dst = AP(
                    out_d,
                    (PAD - dl) * hw + h0 * W,
                    [[W, HC], [-4 * hw, 12], [1, W]],
                )
                src = otile[dl * HC : (dl + 1) * HC, :].rearrange(
                    "p (a w) -> p a w", w=W
                )
                nc.sync.dma_start(dst, src)

        for g in range(ngroups):
            h0 = g * HC
            ltile = lr_pool.tile([C, HC * W], MDT, tag="L")
            rtile = lr_pool.tile([C, HC * W], MDT, tag="R")
            # SWDGE DMA with fp32 -> bf16 cast, split into HCHUNK-row pieces
            for k in range(HC // HCHUNK):
                nc.gpsimd.dma_start(
                    ltile[:, k * HCHUNK * W : (k + 1) * HCHUNK * W].rearrange(
                        "p (a b) -> p a b", a=HCHUNK
                    ),
                    left_d[:, h0 + k * HCHUNK : h0 + (k + 1) * HCHUNK, :],
                )
                nc.gpsimd.dma_start(
                    rtile[:, k * HCHUNK * W : (k + 1) * HCHUNK * W].rearrange(
                        "p (a b) -> p a b", a=HCHUNK
                    ),
                    right_d[:, h0 + k * HCHUNK : h0 + (k + 1) * HCHUNK, :],
                )

            # O spans all 3 w-blocks so out-DMA runs are full 1280B w rows
            otile = o_pool.tile([128, 12 * W], mybir.dt.float32, tag="O")
            stiles = {}

            for wb, m in WBLOCKS:
                ncols, w0 = _ncols(wb, m)
                foff = PAD - wb + w0  # 47 for wb=0 else 0
                # X is h-interleaved: X[i, f*HC + hh] = G_hh[i, f]
                if foff:
                    xtile = x0_tiles[g & 1]
                else:
                    xtile = x_pool.tile([m, HC * FW], SDT, tag="X")

                for hq in range(HC // 4):
                    # 4 matmuls per 2-bank PSUM tile at 256-slot alignment
                    gt = g_pool.tile([m, 1024], mybir.dt.float32, tag="G")
                    for s in range(4):
                        hh = 4 * hq + s
                        nc.tensor.matmul(
                            gt[:, s * 256 : s * 256 + ncols],
                            ltile[:, hh * W + wb : hh * W + wb + m],
                            rtile[:, hh * W + w0 : hh * W + w0 + ncols],
                            start=True,
                            stop=True,
                        )
                    # PSUM -> X with 1/C scaling; dst interleaved (f stride HC).
                    # hh innermost: 4 consecutive bf16 -> packed 32-bit writes
                    dst = AP(
                        xtile[:].tensor,
                        xtile[:].offset + foff * HC + 4 * hq,
                        [[HC * FW, m], [HC, ncols], [1, 4]],
                    )
                    src = AP(gt[:].tensor, gt[:].offset, [[1024, m], [1, ncols], [256, 4]])
                    if copy_parity & 1:
                        nc.scalar.mul(dst, src, 1.0 / C)
                    else:
                        nc.vector.tensor_scalar_mul(dst, src, 1.0 / C)
                    copy_parity += 1

                # shear: S[i, dd*HC + hh] = X[i, (i+dd)*HC + hh]; per-partition
                # window is one contiguous HC*D run -> 2-dim flat AP with a
                # fused (row+byte) step. HW DGE constraints: fused-step APs
                # are only correct with offset 0 and partition count not in
                # {64, 128} -> split m into (m-1) + 1; the single-partition
                # leftover is a plain rectangular DMA.
                stile = s_pool.tile([m, HC * D], SDT, tag=f"S{wb}")
                p_lo = m - 1
                # HWDGE via sync: keeps the shear off the single SWDGE
                # queue so it can't block the input stream behind it, and
                # off the ACT/DVE engines whose FIFOs are busy with copies
                nc.sync.dma_start(
                    stile[0:p_lo, :],
                    AP(
                        xtile[:].tensor,
                        xtile[:].offset,
                        [[HC * (FW + 1), p_lo], [1, HC * D]],
                    ),
                )
                nc.sync.dma_start(
                    stile[p_lo:m, :],
                    xtile[p_lo:m, p_lo * HC : p_lo * HC + HC * D],
                )
                stiles[wb] = stile

            # transposes of the PREVIOUS group run after this group's Gram
            # matmuls so the PE never waits on this group's shear DMAs
            if pending is not None:
                emit_transposes(*pending)
            pending = (stiles, otile, h0)

        emit_transposes(*pending)

    nc.compile()
    return nc


def kernel(left_feature: np.ndarray, right_feature: np.ndarray) -> np.ndarray:
    from concourse import bass_utils

    nc = build_nc()
    ident = np.eye(128, dtype=np.float32)
    in_maps = [
        {
            "left": np.ascontiguousarray(left_feature[b]),
            "right": np.ascontiguousarray(right_feature[b]),
            "ident": ident,
        }
        for b in range(B)
    ]
    res = bass_utils.run_bass_kernel_spmd(nc, in_maps, list(range(B)))
    return np.stack([res.results[b]["out"] for b in range(B)], axis=0)
